# revision 20
# baseline (speedup 1.0000x reference)
"""Trainium2 Bass kernel for CorrelatedSphericalField sampling (v5).

Math (validated against the jax reference):
  coeffs[t] = PHI^t * d_t,   d_t = d_{t-1} + PHI^{-t} * sigma_n (.) xi_{t-1},  d_0 = coeff0
  xs[t,n,k,m] = sum_l d[t,n,l,m] * pct[m,l,k]          (per-m Legendre GEMM)
  out[t,n,k,j] = 4pi * PHI^t * irfft_j(xs), as half-spectrum GEMMs:
      A[.., j] = sum_m xs_re[.., m] C[m, j],  B[.., j] = sum_m xs_im[.., m] S[m, j]
      out[.., 0:362] = A + B ;  out[.., 362+jj] = (A - B)[.., 360-jj]
  PHI^t and 4pi are folded into per-core C/S constants; PHI^{-t}*sigma_n is
  folded into the staged innovations z_t on the host, so the device AR(1)
  prefix is pure adds over contiguous bf16 slabs.

Distribution (8 cores, single launch):
  stages A+B sharded over m (46 of 368 zero-padded m's per core, all (t,n)),
  in 2 m-groups of 16/30 (the gathered m dim is then 128/240 = 128-aligned),
  pipelined with an AllToAll of xs per group (shard dim = t);
  stage D sharded over t (core c handles t=c). The A2A chunk payload is
  m-major so the recv->SBUF reload is a handful of contiguous DMAs.

Data is bf16 end to end (fp32 PSUM accumulation); output returned bf16 and
upcast to fp32 on the host.
"""
import numpy as np
import ml_dtypes

import concourse.bass as bass
import concourse.mybir as mybir
import concourse.tile as tile
from concourse.bass_utils import run_bass_kernel_spmd

# ---- problem constants (hardcoded; kernel must be self-contained) ----
T = 8
N = 16
L = 361          # number of degrees l (contraction dim of stage B)
L2 = 384         # L zero-padded to 3*128
KLAT = 361       # number of latitudes
M = 362          # number of orders m
NLON = 722
JH = 362         # half-spectrum output columns of stage D
NC = 8
MPAD = 368       # M padded to a multiple of NC
MC = MPAD // NC  # 46 m's per core
TN = T * N       # 128
E = 2
S = 8            # AR(1) slabs: [c0, z_1..z_7] (xi[7] is never emitted)

PHI = float(np.exp(-6.0 / 48.0))
FOUR_PI = float(4.0 * np.pi)

LCH = [(0, 128), (128, 256), (256, 384)]
KCH = [(0, 128), (128, 256), (256, 361)]
# A2A m-groups within a core: 16+30 so the packed global m dim is 128 + 240
MGRP = [(0, 16), (16, 46)]
G = len(MGRP)
# packed-m contraction chunks for stage D: (group, row0, rows)
DCH = [(0, 0, 128), (1, 0, 128), (1, 128, 112)]

F32 = mybir.dt.float32
BF16 = mybir.dt.bfloat16
NPBF = ml_dtypes.bfloat16


def _split_multi_waits(nc, max_inline=1):
    """The walrus build in this env accepts only one inline sync-wait per
    instruction; hoist extras onto same-engine NoOps placed just before."""
    ctr = 0
    for f in nc.m.functions:
        for bb in f.blocks:
            new = []
            for inst in bb.instructions:
                si = inst.sync_info
                if si is not None and si.on_wait and len(si.on_wait) > max_inline:
                    waits = list(si.on_wait)
                    keep = waits[-max_inline:]
                    for w in waits[:-max_inline]:
                        ctr += 1
                        nop = mybir.InstNoOp(name=f"I-wsplit-{ctr}",
                                             engine=inst.engine)
                        nop.sync_info = mybir.SyncInfo(on_wait=[w], on_update=[])
                        new.append(nop)
                    inst.sync_info = mybir.SyncInfo(
                        on_wait=keep, on_update=list(si.on_update))
                new.append(inst)
            bb.instructions = new


def build_nc(split_waits=True):
    nc = bass.Bass(num_devices=NC)

    # host layouts:
    #   zc  [l(384), s(8), (m, e, n)]  slab s=0 is c0, s>=1 is PHI^-s sigma xi_{s-1}
    #   pct [m_local(46), l(384), k]
    #   csC/csS [packed-m(368), j] rows permuted to the A2A packed order
    zc_p = nc.declare_dram_parameter("zc_t", [L2, MC * E, S, N], BF16,
                                     isOutput=False)
    pct_p = nc.declare_dram_parameter("pct_t", [MC, L2, KLAT], BF16,
                                      isOutput=False)
    csC_p = nc.declare_dram_parameter("csC", [MPAD, JH], BF16, isOutput=False)
    csS_p = nc.declare_dram_parameter("csS", [MPAD, JH], BF16, isOutput=False)
    out_p = nc.declare_dram_parameter("out_t", [N, KLAT, NLON], BF16,
                                      isOutput=True)

    with tile.TileContext(nc) as tc:
        with tc.tile_pool(name="dram", bufs=1, space="DRAM") as pdram:
            sends, recvs = [], []
            for g, (ga, gb) in enumerate(MGRP):
                mg = gb - ga
                sends.append(pdram.tile([NC, E, mg, N, KLAT], BF16,
                                        name=f"send{g}", tag=f"send{g}"))
                recvs.append(pdram.tile([NC, E, mg, N, KLAT], BF16,
                                        name=f"recv{g}", tag=f"recv{g}"))

            with (
                tc.tile_pool(name="cs", bufs=1) as pcs,
                tc.tile_pool(name="xr", bufs=1) as pxr,
            ):
                # stage-D constants (packed-m chunk rows) + xs gather tiles
                csC_t, csS_t, xr = [], [], {}
                for ch, (gc, r0, rows) in enumerate(DCH):
                    ct = pcs.tile([rows, JH], BF16, name=f"csC{ch}",
                                  tag=f"csC{ch}")
                    st = pcs.tile([rows, JH], BF16, name=f"csS{ch}",
                                  tag=f"csS{ch}")
                    base = 0 if gc == 0 else MGRP[0][1] * NC
                    nc.gpsimd.dma_start(ct[:], csC_p[base + r0:base + r0 + rows])
                    nc.gpsimd.dma_start(st[:], csS_p[base + r0:base + r0 + rows])
                    csC_t.append(ct)
                    csS_t.append(st)
                    for e in range(E):
                        xr[(e, ch)] = pxr.tile([rows, N * KLAT], BF16,
                                               name=f"xr{e}{ch}",
                                               tag=f"xr{e}{ch}")

                with (
                    tc.tile_pool(name="zc", bufs=1) as pzc,
                    tc.tile_pool(name="w", bufs=5) as pw,
                    tc.tile_pool(name="wp", bufs=1) as pwp,
                    tc.tile_pool(name="xs", bufs=6) as pxs,
                    tc.tile_pool(name="psB", bufs=4, space="PSUM") as pp,
                ):
                    # ---- load z/c0 slabs (t-outer), all groups up front ----
                    zc = {}
                    for g, (ga, gb) in enumerate(MGRP):
                        mg = gb - ga
                        for lc, (la, lb) in enumerate(LCH):
                            zt = pzc.tile([128, mg, E, S, N], BF16,
                                          name=f"zc{g}{lc}", tag=f"zc{g}{lc}")
                            zeng = [nc.sync, nc.scalar, nc.gpsimd][lc]
                            zeng.dma_start(
                                zt[:],
                                zc_p[la:lb, ga * E:gb * E])
                            zc[(g, lc)] = zt

                    # ---- stage A: in-place prefix sums over s ----
                    for g in range(G):
                        eng = nc.vector
                        for lc in range(3):
                            zt = zc[(g, lc)]
                            for s in range(S - 1):
                                eng.tensor_tensor(
                                    out=zt[:, :, :, s + 1, :],
                                    in0=zt[:, :, :, s + 1, :],
                                    in1=zt[:, :, :, s, :],
                                    op=mybir.AluOpType.add)

                    # ---- stage B + per-group AllToAll ----
                    # w streams: g0 on sync, g1 on scalar; first 2 pairs of
                    # each group prefetched before the m-loops. gpsimd carries
                    # ONLY the A2A issues (it blocks for the collective's
                    # duration) plus early zc/cs loads.
                    def load_w(mp_, eng, pool=None, tag="pct"):
                        wt = (pool or pw).tile([128, 2, 3, KLAT], BF16, tag=tag)
                        eng.dma_start(
                            wt[:],
                            pct_p[mp_:mp_ + 2].rearrange(
                                "m (c p) k -> p m c k", p=128))
                        return wt

                    wq = {}
                    for mp_ in (0, 2):
                        wq[mp_] = load_w(mp_, nc.sync, pwp, f"pctp{mp_}")
                    for mp_ in (16, 18):
                        wq[mp_] = load_w(mp_, nc.scalar, pwp, f"pctp{mp_}")

                    for g, (ga, gb) in enumerate(MGRP):
                        weng = nc.sync if g == 0 else nc.scalar
                        for mp_ in range(ga, gb, 2):
                            w = wq.pop(mp_) if mp_ in wq else load_w(mp_, weng)
                            for mi in range(2):
                                m = mp_ + mi
                                ml = m - ga
                                xs_sb = pxs.tile([TN, E, KLAT], BF16, tag="xsb")
                                for e in range(E):
                                    ps = pp.tile([TN, KLAT], F32, tag=f"ps{e}")
                                    for lc in range(3):
                                        nc.tensor.matmul(
                                            ps[:],
                                            zc[(g, lc)][:, ml, e],
                                            w[:, mi, lc],
                                            start=(lc == 0), stop=(lc == 2))
                                    nc.scalar.copy(xs_sb[:, e], ps[:])
                                for e in range(E):
                                    nc.sync.dma_start(
                                        sends[g][:, e, ml], xs_sb[:, e])
                    # both A2A issues emitted after ALL stage-B work: no
                    # B-phase DMA is emitted after a collective, so no
                    # cross-queue semaphore threshold can couple B to them;
                    # data deps alone launch each A2A as its sends complete
                    for g in range(G):
                        nc.gpsimd.collective_compute(
                            "AllToAll", mybir.AluOpType.bypass,
                            replica_groups=[list(range(NC))],
                            ins=[sends[g].opt()], outs=[recvs[g].opt()])

                    # xs gather: contiguous m-partition loads per (e, src core),
                    # split where a source core's rows straddle a chunk edge
                    for e in range(E):
                        for gc, (ga, gb) in enumerate(MGRP):
                            mg = gb - ga
                            chunks = [(ci, r0, cnt) for ci, (gg, r0, cnt)
                                      in enumerate(DCH) if gg == gc]
                            for s_ in range(NC):
                                row = s_ * mg  # packed row within this group
                                while row < (s_ + 1) * mg:
                                    ci, r0, cnt = next(
                                        c for c in chunks
                                        if c[1] <= row < c[1] + c[2])
                                    take = min((s_ + 1) * mg, r0 + cnt) - row
                                    (nc.scalar if e == 0
                                     else nc.sync).dma_start(
                                        xr[(e, ci)][row - r0:row - r0 + take],
                                        recvs[gc][s_, e,
                                                  row - s_ * mg:
                                                  row - s_ * mg + take])
                                    row += take

                # ---------------- stage D: iFFT GEMM over packed m ----------
                with (
                    tc.tile_pool(name="o", bufs=6) as po,
                    tc.tile_pool(name="ab", bufs=4) as pab,
                    tc.tile_pool(name="psD", bufs=3, space="PSUM") as pp2,
                ):
                    for n in range(N):
                        for (ka, kb) in KCH:
                            kp = kb - ka
                            psA = pp2.tile([kp, JH], F32, tag="psA")
                            psB = pp2.tile([kp, JH], F32, tag="psB")
                            for ch in range(3):
                                nc.tensor.matmul(
                                    psA[:],
                                    xr[(0, ch)][:, n * KLAT + ka:n * KLAT + kb],
                                    csC_t[ch][:],
                                    start=(ch == 0), stop=(ch == 2))
                            for ch in range(3):
                                nc.tensor.matmul(
                                    psB[:],
                                    xr[(1, ch)][:, n * KLAT + ka:n * KLAT + kb],
                                    csS_t[ch][:],
                                    start=(ch == 0), stop=(ch == 2))
                            a_sb = pab.tile([kp, JH], BF16, tag="a_sb")
                            b_sb = pab.tile([kp, JH], BF16, tag="b_sb")
                            oo = po.tile([kp, NLON], BF16, tag="oo")
                            nc.scalar.copy(a_sb[:], psA[:])
                            nc.vector.tensor_copy(b_sb[:], psB[:])
                            nc.vector.tensor_tensor(
                                out=oo[:, 0:JH], in0=a_sb[:], in1=b_sb[:],
                                op=mybir.AluOpType.add)
                            nc.gpsimd.tensor_tensor(
                                out=oo[:, JH:NLON], in0=a_sb[:, JH - 2:0:-1],
                                in1=b_sb[:, JH - 2:0:-1],
                                op=mybir.AluOpType.subtract)
                            oeng = nc.sync if (n % 2 == 0) else nc.gpsimd
                            oeng.dma_start(out_p[n, ka:kb], oo[:])

    if split_waits:
        _split_multi_waits(nc)
    return nc


def prep_inputs(x, sigma_n, coeff0, xi, pct):
    """Host-side shard/stage: slice + transpose per-core inputs, build
    constants. sigma*PHI^-s is folded into the staged innovation slabs."""
    sigma_n = np.asarray(sigma_n, np.float64)
    coeff0 = np.asarray(coeff0, np.float32)
    xi = np.asarray(xi, np.float32)
    pct = np.asarray(pct, np.float32)

    padm = MPAD - M
    padl = L2 - L

    # zc[l, m, e, s, n]: slab 0 = c0, slab s = PHI^-s * sigma * xi_{s-1}
    zc = np.zeros((L, M, E, S, N), dtype=np.float32)
    zc[:, :, :, 0, :] = np.transpose(coeff0, (1, 2, 3, 0))  # [l, m, e, n]
    phi_inv = PHI ** -(np.arange(1, S, dtype=np.float64))
    # xi: [T, N, L, M, E] -> [l, m, e, s-1, n]
    xi_t = np.transpose(xi[:S - 1], (2, 3, 4, 0, 1))        # [l, m, e, 7, n]
    zc[:, :, :, 1:, :] = xi_t * (
        phi_inv[None, None, None, :, None]
        * sigma_n[:, :, None, None, None]).astype(np.float32)
    zc = np.pad(zc, ((0, padl), (0, padm), (0, 0), (0, 0), (0, 0)))

    pct_pad = np.pad(pct, ((0, padm), (0, padl), (0, 0)))

    # half-spectrum irfft matrices (fp64 host build)
    j = np.arange(JH, dtype=np.float64)
    mm = np.arange(M, dtype=np.float64)
    ang = 2.0 * np.pi * np.outer(mm, j) / NLON
    Cm = 2.0 * np.cos(ang)
    Cm[0, :] = 1.0
    Cm[M - 1, :] = np.cos(np.pi * j)
    Sm = -2.0 * np.sin(ang)
    Sm[0, :] = 0.0
    Sm[M - 1, :] = 0.0
    Cp = np.pad(Cm, ((0, padm), (0, 0)))
    Sp = np.pad(Sm, ((0, padm), (0, 0)))
    # permute rows to the packed A2A order: for each group, core-major
    perm = np.concatenate([
        np.concatenate([np.arange(c * MC + ga, c * MC + gb) for c in range(NC)])
        for (ga, gb) in MGRP])
    Cp = Cp[perm]
    Sp = Sp[perm]

    in_maps = []
    for c in range(NC):
        msl = slice(c * MC, (c + 1) * MC)
        zc_c = np.ascontiguousarray(zc[:, msl]).reshape(
            L2, MC * E, S, N).astype(NPBF)
        pct_c = np.ascontiguousarray(pct_pad[msl]).astype(NPBF)
        scale = FOUR_PI * PHI ** c
        in_maps.append({
            "zc_t": zc_c,
            "pct_t": pct_c,
            "csC": (scale * Cp).astype(NPBF),
            "csS": (scale * Sp).astype(NPBF),
        })
    return in_maps


_NC_CACHE = None


def kernel(x, sigma_n, coeff0, xi, pct):
    global _NC_CACHE
    in_maps = prep_inputs(x, sigma_n, coeff0, xi, pct)
    if _NC_CACHE is None:
        _NC_CACHE = build_nc()
    res = run_bass_kernel_spmd(_NC_CACHE, in_maps, list(range(NC)))
    out = np.stack([np.asarray(res.results[c]["out_t"], dtype=np.float32)
                    for c in range(NC)], axis=0)
    return out.reshape(T, 1, 1, N, KLAT, NLON)


# revision 21
# speedup vs baseline: 1.0377x; 1.0377x over previous
"""Trainium2 Bass kernel for CorrelatedSphericalField sampling (v5).

Math (validated against the jax reference):
  coeffs[t] = PHI^t * d_t,   d_t = d_{t-1} + PHI^{-t} * sigma_n (.) xi_{t-1},  d_0 = coeff0
  xs[t,n,k,m] = sum_l d[t,n,l,m] * pct[m,l,k]          (per-m Legendre GEMM)
  out[t,n,k,j] = 4pi * PHI^t * irfft_j(xs), as half-spectrum GEMMs:
      A[.., j] = sum_m xs_re[.., m] C[m, j],  B[.., j] = sum_m xs_im[.., m] S[m, j]
      out[.., 0:362] = A + B ;  out[.., 362+jj] = (A - B)[.., 360-jj]
  PHI^t and 4pi are folded into per-core C/S constants; PHI^{-t}*sigma_n is
  folded into the staged innovations z_t on the host, so the device AR(1)
  prefix is pure adds over contiguous bf16 slabs.

Distribution (8 cores, single launch):
  stages A+B sharded over m (46 of 368 zero-padded m's per core, all (t,n)),
  in 2 m-groups of 16/30 (the gathered m dim is then 128/240 = 128-aligned),
  pipelined with an AllToAll of xs per group (shard dim = t);
  stage D sharded over t (core c handles t=c). The A2A chunk payload is
  m-major so the recv->SBUF reload is a handful of contiguous DMAs.

Data is bf16 end to end (fp32 PSUM accumulation); output returned bf16 and
upcast to fp32 on the host.
"""
import numpy as np
import ml_dtypes

import concourse.bass as bass
import concourse.mybir as mybir
import concourse.tile as tile
from concourse.bass_utils import run_bass_kernel_spmd

# ---- problem constants (hardcoded; kernel must be self-contained) ----
T = 8
N = 16
L = 361          # number of degrees l (contraction dim of stage B)
L2 = 384         # L zero-padded to 3*128
KLAT = 361       # number of latitudes
M = 362          # number of orders m
NLON = 722
JH = 362         # half-spectrum output columns of stage D
NC = 8
MPAD = 368       # M padded to a multiple of NC
MC = MPAD // NC  # 46 m's per core
TN = T * N       # 128
E = 2
S = 8            # AR(1) slabs: [c0, z_1..z_7] (xi[7] is never emitted)

PHI = float(np.exp(-6.0 / 48.0))
FOUR_PI = float(4.0 * np.pi)

LCH = [(0, 128), (128, 256), (256, 384)]
KCH = [(0, 128), (128, 256), (256, 361)]
# A2A m-groups within a core: 16+30 so the packed global m dim is 128 + 240
MGRP = [(0, 16), (16, 46)]
G = len(MGRP)
# packed-m contraction chunks for stage D: (group, row0, rows)
DCH = [(0, 0, 128), (1, 0, 128), (1, 128, 112)]

F32 = mybir.dt.float32
BF16 = mybir.dt.bfloat16
NPBF = ml_dtypes.bfloat16


def _split_multi_waits(nc, max_inline=1):
    """The walrus build in this env accepts only one inline sync-wait per
    instruction; hoist extras onto same-engine NoOps placed just before."""
    ctr = 0
    for f in nc.m.functions:
        for bb in f.blocks:
            new = []
            for inst in bb.instructions:
                si = inst.sync_info
                if si is not None and si.on_wait and len(si.on_wait) > max_inline:
                    waits = list(si.on_wait)
                    keep = waits[-max_inline:]
                    for w in waits[:-max_inline]:
                        ctr += 1
                        nop = mybir.InstNoOp(name=f"I-wsplit-{ctr}",
                                             engine=inst.engine)
                        nop.sync_info = mybir.SyncInfo(on_wait=[w], on_update=[])
                        new.append(nop)
                    inst.sync_info = mybir.SyncInfo(
                        on_wait=keep, on_update=list(si.on_update))
                new.append(inst)
            bb.instructions = new


def build_nc(split_waits=True):
    nc = bass.Bass(num_devices=NC)

    # host layouts:
    #   zc  [l(384), s(8), (m, e, n)]  slab s=0 is c0, s>=1 is PHI^-s sigma xi_{s-1}
    #   pct [m_local(46), l(384), k]
    #   csC/csS [packed-m(368), j] rows permuted to the A2A packed order
    zc_p = nc.declare_dram_parameter("zc_t", [L2, MC * E, S, N], BF16,
                                     isOutput=False)
    pct_p = nc.declare_dram_parameter("pct_t", [128, MC, 3, KLAT], BF16,
                                      isOutput=False)
    csC_p = nc.declare_dram_parameter("csC", [MPAD, JH], BF16, isOutput=False)
    csS_p = nc.declare_dram_parameter("csS", [MPAD, JH], BF16, isOutput=False)
    out_p = nc.declare_dram_parameter("out_t", [N, KLAT, NLON], BF16,
                                      isOutput=True)

    with tile.TileContext(nc) as tc:
        with tc.tile_pool(name="dram", bufs=1, space="DRAM") as pdram:
            sends, recvs = [], []
            for g, (ga, gb) in enumerate(MGRP):
                mg = gb - ga
                sends.append(pdram.tile([NC, E, mg, N, KLAT], BF16,
                                        name=f"send{g}", tag=f"send{g}"))
                recvs.append(pdram.tile([NC, E, mg, N, KLAT], BF16,
                                        name=f"recv{g}", tag=f"recv{g}"))

            with (
                tc.tile_pool(name="cs", bufs=1) as pcs,
                tc.tile_pool(name="xr", bufs=1) as pxr,
            ):
                # stage-D constants (packed-m chunk rows) + xs gather tiles
                csC_t, csS_t, xr = [], [], {}
                for ch, (gc, r0, rows) in enumerate(DCH):
                    ct = pcs.tile([rows, JH], BF16, name=f"csC{ch}",
                                  tag=f"csC{ch}")
                    st = pcs.tile([rows, JH], BF16, name=f"csS{ch}",
                                  tag=f"csS{ch}")
                    base = 0 if gc == 0 else MGRP[0][1] * NC
                    nc.gpsimd.dma_start(ct[:], csC_p[base + r0:base + r0 + rows])
                    nc.gpsimd.dma_start(st[:], csS_p[base + r0:base + r0 + rows])
                    csC_t.append(ct)
                    csS_t.append(st)
                    for e in range(E):
                        xr[(e, ch)] = pxr.tile([rows, N * KLAT], BF16,
                                               name=f"xr{e}{ch}",
                                               tag=f"xr{e}{ch}")

                with (
                    tc.tile_pool(name="zc", bufs=1) as pzc,
                    tc.tile_pool(name="w", bufs=5) as pw,
                    tc.tile_pool(name="wp", bufs=1) as pwp,
                    tc.tile_pool(name="xs", bufs=6) as pxs,
                    tc.tile_pool(name="psB", bufs=4, space="PSUM") as pp,
                ):
                    # ---- load z/c0 slabs (t-outer), all groups up front ----
                    zc = {}
                    for g, (ga, gb) in enumerate(MGRP):
                        mg = gb - ga
                        for lc, (la, lb) in enumerate(LCH):
                            zt = pzc.tile([128, mg, E, S, N], BF16,
                                          name=f"zc{g}{lc}", tag=f"zc{g}{lc}")
                            zeng = [nc.sync, nc.scalar, nc.gpsimd][lc]
                            zeng.dma_start(
                                zt[:],
                                zc_p[la:lb, ga * E:gb * E])
                            zc[(g, lc)] = zt

                    # ---- stage A: in-place prefix sums over s ----
                    for g in range(G):
                        eng = nc.vector
                        for lc in range(3):
                            zt = zc[(g, lc)]
                            for s in range(S - 1):
                                eng.tensor_tensor(
                                    out=zt[:, :, :, s + 1, :],
                                    in0=zt[:, :, :, s + 1, :],
                                    in1=zt[:, :, :, s, :],
                                    op=mybir.AluOpType.add)

                    # ---- stage B + per-group AllToAll ----
                    # w streams: g0 on sync, g1 on scalar; first 2 pairs of
                    # each group prefetched before the m-loops. gpsimd carries
                    # ONLY the A2A issues (it blocks for the collective's
                    # duration) plus early zc/cs loads.
                    def load_w(mp_, eng, pool=None, tag="pct"):
                        wt = (pool or pw).tile([128, 2, 3, KLAT], BF16, tag=tag)
                        eng.dma_start(
                            wt[:], pct_p[:, mp_:mp_ + 2])
                        return wt

                    wq = {}
                    for mp_ in (0, 2):
                        wq[mp_] = load_w(mp_, nc.sync, pwp, f"pctp{mp_}")
                    for mp_ in (16, 18):
                        wq[mp_] = load_w(mp_, nc.scalar, pwp, f"pctp{mp_}")

                    for g, (ga, gb) in enumerate(MGRP):
                        weng = nc.sync if g == 0 else nc.scalar
                        for mp_ in range(ga, gb, 2):
                            w = wq.pop(mp_) if mp_ in wq else load_w(mp_, weng)
                            for mi in range(2):
                                m = mp_ + mi
                                ml = m - ga
                                xs_sb = pxs.tile([TN, E, KLAT], BF16, tag="xsb")
                                for e in range(E):
                                    ps = pp.tile([TN, KLAT], F32, tag=f"ps{e}")
                                    for lc in range(3):
                                        nc.tensor.matmul(
                                            ps[:],
                                            zc[(g, lc)][:, ml, e],
                                            w[:, mi, lc],
                                            start=(lc == 0), stop=(lc == 2))
                                    nc.scalar.copy(xs_sb[:, e], ps[:])
                                for e in range(E):
                                    nc.sync.dma_start(
                                        sends[g][:, e, ml], xs_sb[:, e])
                    # both A2A issues emitted after ALL stage-B work: no
                    # B-phase DMA is emitted after a collective, so no
                    # cross-queue semaphore threshold can couple B to them;
                    # data deps alone launch each A2A as its sends complete
                    for g in range(G):
                        nc.gpsimd.collective_compute(
                            "AllToAll", mybir.AluOpType.bypass,
                            replica_groups=[list(range(NC))],
                            ins=[sends[g].opt()], outs=[recvs[g].opt()])

                    # xs gather: contiguous m-partition loads per (e, src core),
                    # split where a source core's rows straddle a chunk edge
                    for e in range(E):
                        for gc, (ga, gb) in enumerate(MGRP):
                            mg = gb - ga
                            chunks = [(ci, r0, cnt) for ci, (gg, r0, cnt)
                                      in enumerate(DCH) if gg == gc]
                            for s_ in range(NC):
                                row = s_ * mg  # packed row within this group
                                while row < (s_ + 1) * mg:
                                    ci, r0, cnt = next(
                                        c for c in chunks
                                        if c[1] <= row < c[1] + c[2])
                                    take = min((s_ + 1) * mg, r0 + cnt) - row
                                    (nc.scalar if e == 0
                                     else nc.sync).dma_start(
                                        xr[(e, ci)][row - r0:row - r0 + take],
                                        recvs[gc][s_, e,
                                                  row - s_ * mg:
                                                  row - s_ * mg + take])
                                    row += take

                # ---------------- stage D: iFFT GEMM over packed m ----------
                with (
                    tc.tile_pool(name="o", bufs=6) as po,
                    tc.tile_pool(name="ab", bufs=4) as pab,
                    tc.tile_pool(name="psD", bufs=3, space="PSUM") as pp2,
                ):
                    for n in range(N):
                        for (ka, kb) in KCH:
                            kp = kb - ka
                            psA = pp2.tile([kp, JH], F32, tag="psA")
                            psB = pp2.tile([kp, JH], F32, tag="psB")
                            for ch in range(3):
                                nc.tensor.matmul(
                                    psA[:],
                                    xr[(0, ch)][:, n * KLAT + ka:n * KLAT + kb],
                                    csC_t[ch][:],
                                    start=(ch == 0), stop=(ch == 2))
                            for ch in range(3):
                                nc.tensor.matmul(
                                    psB[:],
                                    xr[(1, ch)][:, n * KLAT + ka:n * KLAT + kb],
                                    csS_t[ch][:],
                                    start=(ch == 0), stop=(ch == 2))
                            a_sb = pab.tile([kp, JH], BF16, tag="a_sb")
                            b_sb = pab.tile([kp, JH], BF16, tag="b_sb")
                            oo = po.tile([kp, NLON], BF16, tag="oo")
                            nc.scalar.copy(a_sb[:], psA[:])
                            nc.vector.tensor_copy(b_sb[:], psB[:])
                            nc.vector.tensor_tensor(
                                out=oo[:, 0:JH], in0=a_sb[:], in1=b_sb[:],
                                op=mybir.AluOpType.add)
                            nc.gpsimd.tensor_tensor(
                                out=oo[:, JH:NLON], in0=a_sb[:, JH - 2:0:-1],
                                in1=b_sb[:, JH - 2:0:-1],
                                op=mybir.AluOpType.subtract)
                            oeng = nc.sync if (n % 2 == 0) else nc.gpsimd
                            oeng.dma_start(out_p[n, ka:kb], oo[:])

    if split_waits:
        _split_multi_waits(nc)
    return nc


def prep_inputs(x, sigma_n, coeff0, xi, pct):
    """Host-side shard/stage: slice + transpose per-core inputs, build
    constants. sigma*PHI^-s is folded into the staged innovation slabs."""
    sigma_n = np.asarray(sigma_n, np.float64)
    coeff0 = np.asarray(coeff0, np.float32)
    xi = np.asarray(xi, np.float32)
    pct = np.asarray(pct, np.float32)

    padm = MPAD - M
    padl = L2 - L

    # zc[l, m, e, s, n]: slab 0 = c0, slab s = PHI^-s * sigma * xi_{s-1}
    zc = np.zeros((L, M, E, S, N), dtype=np.float32)
    zc[:, :, :, 0, :] = np.transpose(coeff0, (1, 2, 3, 0))  # [l, m, e, n]
    phi_inv = PHI ** -(np.arange(1, S, dtype=np.float64))
    # xi: [T, N, L, M, E] -> [l, m, e, s-1, n]
    xi_t = np.transpose(xi[:S - 1], (2, 3, 4, 0, 1))        # [l, m, e, 7, n]
    zc[:, :, :, 1:, :] = xi_t * (
        phi_inv[None, None, None, :, None]
        * sigma_n[:, :, None, None, None]).astype(np.float32)
    zc = np.pad(zc, ((0, padl), (0, padm), (0, 0), (0, 0), (0, 0)))

    pct_pad = np.pad(pct, ((0, padm), (0, padl), (0, 0)))

    # half-spectrum irfft matrices (fp64 host build)
    j = np.arange(JH, dtype=np.float64)
    mm = np.arange(M, dtype=np.float64)
    ang = 2.0 * np.pi * np.outer(mm, j) / NLON
    Cm = 2.0 * np.cos(ang)
    Cm[0, :] = 1.0
    Cm[M - 1, :] = np.cos(np.pi * j)
    Sm = -2.0 * np.sin(ang)
    Sm[0, :] = 0.0
    Sm[M - 1, :] = 0.0
    Cp = np.pad(Cm, ((0, padm), (0, 0)))
    Sp = np.pad(Sm, ((0, padm), (0, 0)))
    # permute rows to the packed A2A order: for each group, core-major
    perm = np.concatenate([
        np.concatenate([np.arange(c * MC + ga, c * MC + gb) for c in range(NC)])
        for (ga, gb) in MGRP])
    Cp = Cp[perm]
    Sp = Sp[perm]

    in_maps = []
    for c in range(NC):
        msl = slice(c * MC, (c + 1) * MC)
        zc_c = np.ascontiguousarray(zc[:, msl]).reshape(
            L2, MC * E, S, N).astype(NPBF)
        pct_c = np.ascontiguousarray(
            pct_pad[msl].reshape(MC, 3, 128, KLAT).transpose(2, 0, 1, 3)
        ).astype(NPBF)
        scale = FOUR_PI * PHI ** c
        in_maps.append({
            "zc_t": zc_c,
            "pct_t": pct_c,
            "csC": (scale * Cp).astype(NPBF),
            "csS": (scale * Sp).astype(NPBF),
        })
    return in_maps


_NC_CACHE = None


def kernel(x, sigma_n, coeff0, xi, pct):
    global _NC_CACHE
    in_maps = prep_inputs(x, sigma_n, coeff0, xi, pct)
    if _NC_CACHE is None:
        _NC_CACHE = build_nc()
    res = run_bass_kernel_spmd(_NC_CACHE, in_maps, list(range(NC)))
    out = np.stack([np.asarray(res.results[c]["out_t"], dtype=np.float32)
                    for c in range(NC)], axis=0)
    return out.reshape(T, 1, 1, N, KLAT, NLON)
